# revision 15
# baseline (speedup 1.0000x reference)
"""CCPL loss kernel for Trainium2, 8 NeuronCores, SPMD data-parallel over (batch, S-half).

Self-contained: takes the full unsharded inputs (as produced by the reference
setup_inputs), shards across 8 cores, runs one Bass/Tile program per core,
and reduces the per-core partial sums on the host.
"""
import sys
import numpy as np

sys.path.insert(0, "/opt/trn_rl_repo")

from contextlib import ExitStack

import concourse.bass as bass
import concourse.tile as tile
from concourse import bacc, mybir, bass_utils
from concourse import masks

F32 = mybir.dt.float32
F32R = mybir.dt.float32r
I16 = mybir.dt.int16
AF = mybir.ActivationFunctionType
ALU = mybir.AluOpType

B = 4
NUM_S = 4096            # neighbor pairs per layer (S)
HALF = 2048             # rows per core
TAU = 0.01
INVTAU = 100.0
LAYERS = [(64, 256 * 256), (128, 128 * 128), (256, 64 * 64), (512, 32 * 32)]  # (C, HW)
IDX_BLK = 1024          # idxs per dma_gather (512 c + 512 n)
DBLK = 512              # d-rows per block
QBLOCKS = 4             # 4*512 = 2048 q rows
KBLOCKS = 8             # 8*512 = 4096 k rows
NBLOCKS = QBLOCKS + KBLOCKS


def _gsrc_shape(i):
    C, HW = LAYERS[i]
    if i == 0:
        return [HW // 2, 2 * C]      # paired rows, idx>>1
    return [HW, C]


def build_bass(layers=(0, 1, 2, 3), do_nce=True, do_mlp=True, do_lp=True, nstrips=16):
    nc = bacc.Bacc("TRN2", target_bir_lowering=False, debug=False)

    # ---- DRAM tensors ----
    gq, gk, w0t, w1t, w2t, b0d, b1d, b2d, qi, ki = {}, {}, {}, {}, {}, {}, {}, {}, {}, {}
    o_negm, o_ssum, o_lp = {}, {}, {}
    for i, (C, HW) in enumerate(LAYERS):
        Cout = C // 4
        gq[i] = nc.dram_tensor(f"gq{i}", _gsrc_shape(i), F32, kind="ExternalInput").ap()
        gk[i] = nc.dram_tensor(f"gk{i}", _gsrc_shape(i), F32, kind="ExternalInput").ap()
        w0t[i] = nc.dram_tensor(f"w0t{i}", [C, C], F32R, kind="ExternalInput").ap()
        w1t[i] = nc.dram_tensor(f"w1t{i}", [C, C], F32R, kind="ExternalInput").ap()
        w2t[i] = nc.dram_tensor(f"w2t{i}", [C, Cout], F32R, kind="ExternalInput").ap()
        b0d[i] = nc.dram_tensor(f"b0_{i}", [C, 1], F32, kind="ExternalInput").ap()
        b1d[i] = nc.dram_tensor(f"b1_{i}", [C, 1], F32, kind="ExternalInput").ap()
        b2d[i] = nc.dram_tensor(f"b2_{i}", [Cout, 1], F32, kind="ExternalInput").ap()
        qi[i] = nc.dram_tensor(f"qi{i}", [128, QBLOCKS * IDX_BLK // 16], I16, kind="ExternalInput").ap()
        ki[i] = nc.dram_tensor(f"ki{i}", [128, KBLOCKS * IDX_BLK // 16], I16, kind="ExternalInput").ap()
        o_negm[i] = nc.dram_tensor(f"negm{i}", [128, 16], F32, kind="ExternalOutput").ap()
        o_ssum[i] = nc.dram_tensor(f"ssum{i}", [128, 16], F32, kind="ExternalOutput").ap()
        o_lp[i] = nc.dram_tensor(f"lp{i}", [Cout, 1], F32, kind="ExternalOutput").ap()
    qm0 = nc.dram_tensor("qm0", [128, QBLOCKS * 512], F32, kind="ExternalInput").ap()
    km0 = nc.dram_tensor("km0", [128, KBLOCKS * 512], F32, kind="ExternalInput").ap()

    with tile.TileContext(nc) as tc, ExitStack() as ctx:
        const_pool = ctx.enter_context(tc.tile_pool(name="const", bufs=1))
        wpool = ctx.enter_context(tc.tile_pool(name="w", bufs=1))
        ipool = ctx.enter_context(tc.tile_pool(name="idx", bufs=1))
        gpool = ctx.enter_context(tc.tile_pool(name="gather", bufs=2))
        selpool = ctx.enter_context(tc.tile_pool(name="sel", bufs=2))
        dpool = ctx.enter_context(tc.tile_pool(name="dT", bufs=2))
        xpool = ctx.enter_context(tc.tile_pool(name="x", bufs=2))
        ypool = ctx.enter_context(tc.tile_pool(name="y", bufs=2))
        obuf = ctx.enter_context(tc.tile_pool(name="obuf", bufs=2))
        tinyp = ctx.enter_context(tc.tile_pool(name="tiny", bufs=4))
        scrp = ctx.enter_context(tc.tile_pool(name="scr", bufs=1))
        tpsum = ctx.enter_context(tc.tile_pool(name="tps", bufs=2, space="PSUM"))
        mpsum = ctx.enter_context(tc.tile_pool(name="mps", bufs=2, space="PSUM"))
        npsum = ctx.enter_context(tc.tile_pool(name="nps", bufs=2, space="PSUM"))

        ident = const_pool.tile([128, 128], F32)
        masks.make_identity(nc, ident[:])

        # ---- load weights / biases / idxs ----
        wsb = {}
        bsb = {}
        isb = {}
        for i, (C, HW) in enumerate(LAYERS):
            Cout = C // 4
            CB = (C + 127) // 128
            for j, wd, cols in ((0, w0t[i], C), (1, w1t[i], C), (2, w2t[i], Cout)):
                tiles = []
                for cb in range(CB):
                    cw = min(128, C - cb * 128)
                    t = wpool.tile([128, cols], F32R, tag=f"w{j}_{i}_{cb}")
                    nc.sync.dma_start(t[:cw, :], wd[cb * 128: cb * 128 + cw, :])
                    tiles.append(t)
                wsb[(i, j)] = tiles
            for j, bd, rows in ((0, b0d[i], C), (1, b1d[i], C), (2, b2d[i], Cout)):
                t = wpool.tile([128, (rows + 127) // 128], F32, tag=f"b{j}_{i}")
                bt = t[:].rearrange("p (cb o) -> p cb o", o=1)
                for cb in range((rows + 127) // 128):
                    cw = min(128, rows - cb * 128)
                    nc.sync.dma_start(bt[:cw, cb, :], bd[cb * 128: cb * 128 + cw, :])
                bsb[(i, j)] = bt
            tq = ipool.tile([128, QBLOCKS * IDX_BLK // 16], I16, tag=f"qi{i}")
            nc.sync.dma_start(tq[:], qi[i])
            tk = ipool.tile([128, KBLOCKS * IDX_BLK // 16], I16, tag=f"ki{i}")
            nc.sync.dma_start(tk[:], ki[i])
            isb[i] = (tq, tk)
        m0sb = ipool.tile([128, QBLOCKS * 512], F32, tag="qm0")
        nc.sync.dma_start(m0sb[:], qm0)
        m0sbk = ipool.tile([128, KBLOCKS * 512], F32, tag="km0")
        nc.sync.dma_start(m0sbk[:], km0)

        # ---- per-layer pipeline ----
        for i, (C, HW) in enumerate(LAYERS):
            if i not in layers:
                continue
            Cout = C // 4
            CB = (C + 127) // 128
            ELEM = 2 * C if i == 0 else C
            JB = IDX_BLK // 128          # 8 j-blocks per gather block
            y = ypool.tile([128, NBLOCKS * DBLK], F32R, tag="y")

            for g in range(NBLOCKS):
                is_q = g < QBLOCKS
                src = gq[i] if is_q else gk[i]
                itile = isb[i][0] if is_q else isb[i][1]
                gg = g if is_q else g - QBLOCKS
                icols = slice(gg * (IDX_BLK // 16), (gg + 1) * (IDX_BLK // 16))

                gt = gpool.tile([128, JB * ELEM], F32, tag="gt")
                gt3 = gt[:].rearrange("p (j c) -> p j c", j=JB)
                nc.gpsimd.dma_gather(
                    out_ap=gt3, in_ap=src, idxs_ap=itile[:, icols],
                    num_idxs=IDX_BLK, num_idxs_reg=IDX_BLK,
                    elem_size=ELEM, transpose=False,
                )

                if i == 0:
                    msk = m0sb if is_q else m0sbk
                    mflat = msk[:, gg * 512:(gg + 1) * 512]
                    sel = selpool.tile([128, JB * 64], F32, tag="sel")
                    sel3 = sel[:].rearrange("p (j c) -> p j c", j=JB)
                    tmp = selpool.tile([128, JB * 64], F32, tag="selt")
                    # sel = odd + (even - odd) * mask   (mask=1 -> even row)
                    nc.vector.tensor_sub(tmp[:], gt3[:, :, 0:64], gt3[:, :, 64:128])
                    nc.vector.tensor_mul(tmp[:], tmp[:], mflat)
                    nc.vector.tensor_add(sel3, tmp[:], gt3[:, :, 64:128])
                    src_t = sel3
                else:
                    src_t = gt3

                # row-major d = c - n, then PE-transpose to channel-major
                CW = 64 if i == 0 else C
                drow = selpool.tile([128, 4 * CW], F32, tag="drow")
                nc.vector.tensor_sub(drow[:], src_t[:, 0:4, :], src_t[:, 4:8, :])
                drow3 = drow[:].rearrange("p (j c) -> p j c", j=4)
                dT = dpool.tile([128, CB * DBLK], F32R, tag="dT")
                for cb in range(CB):
                    cw = min(128, C - cb * 128)
                    ps = tpsum.tile([128, DBLK], F32, tag="tps")
                    for j in range(4):
                        nc.tensor.matmul(
                            ps[:cw, j * 128:(j + 1) * 128],
                            drow3[:, j, cb * 128: cb * 128 + cw],
                            ident[:], is_transpose=True, start=True, stop=True)
                    nc.vector.tensor_copy(dT[:cw, cb * DBLK:(cb + 1) * DBLK], ps[:cw, :])

                if not do_mlp:
                    continue
                # MLP: x1 = relu(W0 d + b0); x2 = relu(W1 x1 + b1); y = W2 x2 + b2
                xin = dT
                for j in range(2):
                    xout = xpool.tile([128, CB * DBLK], F32R, tag="x")
                    wt = wsb[(i, j)]
                    bt = bsb[(i, j)]
                    for cbo in range(CB):
                        cwo = min(128, C - cbo * 128)
                        ps = mpsum.tile([128, DBLK], F32, tag="mps")
                        for cbi in range(CB):
                            cwi = min(128, C - cbi * 128)
                            nc.tensor.matmul(
                                ps[:cwo, :],
                                wt[cbi][:cwi, cbo * 128: cbo * 128 + cwo],
                                xin[:cwi, cbi * DBLK:(cbi + 1) * DBLK],
                                start=(cbi == 0), stop=(cbi == CB - 1))
                        dst = xout[:cwo, cbo * DBLK:(cbo + 1) * DBLK]
                        if (g + j + cbo) % 2 == 0:
                            nc.scalar.activation(dst, ps[:cwo, :], AF.Relu,
                                                 bias=bt[:cwo, cbo, :], scale=1.0)
                        else:
                            nc.vector.tensor_scalar(dst, ps[:cwo, :], bt[:cwo, cbo, :],
                                                    0.0, op0=ALU.add, op1=ALU.max)
                    xin = xout
                # final linear -> y block
                ps = mpsum.tile([128, DBLK], F32, tag="mps")
                wt = wsb[(i, 2)]
                for cbi in range(CB):
                    cwi = min(128, C - cbi * 128)
                    nc.tensor.matmul(ps[:Cout, :], wt[cbi][:cwi, :Cout],
                                     xin[:cwi, cbi * DBLK:(cbi + 1) * DBLK],
                                     start=(cbi == 0), stop=(cbi == CB - 1))
                nc.scalar.activation(y[:Cout, g * DBLK:(g + 1) * DBLK], ps[:Cout, :],
                                     AF.Identity, bias=bsb[(i, 2)][:Cout, 0, :], scale=1.0)

            # ---- NCE over y: q = y[:, :2048], k = y[:, 2048:6144] ----
            if not (do_nce and do_mlp):
                continue
            yq = y[:Cout, 0:HALF]
            yk_off = HALF
            lp = obuf.tile([128, 1], F32, tag="lp")
            if do_lp:
                lp_scr = scrp.tile([128, HALF], F32, tag="lpscr")
                nc.vector.tensor_mul(lp_scr[:Cout, :], yq.bitcast(F32),
                                     y[:Cout, yk_off:yk_off + HALF].bitcast(F32))
                nc.vector.tensor_reduce(lp[:Cout, :], lp_scr[:Cout, :],
                                        axis=mybir.AxisListType.X, op=ALU.add)
            negmbuf = obuf.tile([128, 16], F32, tag="negm")
            ssumbuf = obuf.tile([128, 16], F32, tag="ssum")
            for m in range(nstrips):
                lhs = y[:Cout, m * 128:(m + 1) * 128]
                mxq = tinyp.tile([128, 4], F32, tag="mxq")
                for qt in range(4):
                    ps = npsum.tile([128, 1024], F32, tag="nps")
                    for nn in range(2):
                        nc.tensor.matmul(
                            ps[:, nn * 512:(nn + 1) * 512], lhs,
                            y[:Cout, yk_off + qt * 1024 + nn * 512: yk_off + qt * 1024 + (nn + 1) * 512],
                            start=True, stop=True)
                    nc.vector.tensor_reduce(mxq[:, qt:qt + 1], ps[:], axis=mybir.AxisListType.X, op=ALU.max)
                mx = tinyp.tile([128, 1], F32, tag="mx")
                nc.vector.tensor_reduce(mx[:], mxq[:], axis=mybir.AxisListType.X, op=ALU.max)
                nc.vector.tensor_scalar(negmbuf[:, m:m + 1], mx[:], -INVTAU, None, op0=ALU.mult)
                accq = tinyp.tile([128, 4], F32, tag="accq")
                for qt in range(4):
                    ps = npsum.tile([128, 1024], F32, tag="nps")
                    for nn in range(2):
                        nc.tensor.matmul(
                            ps[:, nn * 512:(nn + 1) * 512], lhs,
                            y[:Cout, yk_off + qt * 1024 + nn * 512: yk_off + qt * 1024 + (nn + 1) * 512],
                            start=True, stop=True)
                    nc.scalar.activation(ps[:], ps[:], AF.Exp,
                                         bias=negmbuf[:, m:m + 1], scale=INVTAU,
                                         accum_out=accq[:, qt:qt + 1])
                nc.vector.tensor_reduce(ssumbuf[:, m:m + 1], accq[:], axis=mybir.AxisListType.X, op=ALU.add)
            if nstrips:
                nc.sync.dma_start(o_negm[i][:, :nstrips], negmbuf[:, :nstrips])
                nc.sync.dma_start(o_ssum[i][:, :nstrips], ssumbuf[:, :nstrips])
            if do_lp:
                nc.sync.dma_start(o_lp[i], lp[:Cout, :])

    nc.compile()
    return nc


def _wrap_idx(idx):
    n = idx.shape[0]
    w = np.ascontiguousarray(idx.reshape(n // 16, 16).T.astype(np.int16))
    return np.ascontiguousarray(np.tile(w, (8, 1)))


def _expand_mask(par):
    # par: [n] float32 (1.0 = take even half); -> [128, (n//128)*64]
    m = np.ascontiguousarray(par.reshape(-1, 128).T)          # [128, n/128]
    m = np.repeat(m[:, :, None], 64, axis=2)
    return np.ascontiguousarray(m.reshape(128, -1).astype(np.float32))


def _block_interleave(c_list, n_list):
    # -> [c0 n0 c1 n1 ...] with 512-element sub-blocks
    out = []
    for g in range(len(c_list) // 512):
        out.append(c_list[g * 512:(g + 1) * 512])
        out.append(n_list[g * 512:(g + 1) * 512])
    return np.concatenate(out)


def prep_in_maps(inputs):
    inp = {k: np.asarray(v) for k, v in inputs.items()}
    shared = {}
    for i, (C, HW) in enumerate(LAYERS):
        for b in range(B):
            for nm, key in (("gq", f"fq{i}"), ("gk", f"fk{i}")):
                f = inp[key][b].reshape(C, HW).T  # [HW, C]
                f = np.ascontiguousarray(f.astype(np.float32))
                if i == 0:
                    f = f.reshape(HW // 2, 2 * C)
                shared[(nm, i, b)] = f
        shared[("w0t", i)] = np.ascontiguousarray(inp[f"w{i}_0"].T.astype(np.float32))
        shared[("w1t", i)] = np.ascontiguousarray(inp[f"w{i}_1"].T.astype(np.float32))
        shared[("w2t", i)] = np.ascontiguousarray(inp[f"w{i}_2"].T.astype(np.float32))
        for j in range(3):
            shared[(f"b{j}", i)] = np.ascontiguousarray(
                inp[f"b{i}_{j}"].astype(np.float32)[:, None])

    in_maps = []
    for core in range(8):
        b, h = core // 2, core % 2
        im = {}
        for i, (C, HW) in enumerate(LAYERS):
            cid = np.asarray(inp[f"cid{i}"]).astype(np.int64)
            nid = np.asarray(inp[f"nid{i}"]).astype(np.int64)
            c_h, n_h = cid[h * HALF:(h + 1) * HALF], nid[h * HALF:(h + 1) * HALF]
            c_o, n_o = cid[(1 - h) * HALF:(2 - h) * HALF], nid[(1 - h) * HALF:(2 - h) * HALF]
            q_list = _block_interleave(c_h, n_h)
            k_list = _block_interleave(np.concatenate([c_h, c_o]), np.concatenate([n_h, n_o]))
            if i == 0:
                im["qm0"] = _expand_mask((1 - (q_list & 1)).astype(np.float32))
                im["km0"] = _expand_mask((1 - (k_list & 1)).astype(np.float32))
                q_list, k_list = q_list >> 1, k_list >> 1
            im[f"qi{i}"] = _wrap_idx(q_list)
            im[f"ki{i}"] = _wrap_idx(k_list)
            im[f"gq{i}"] = shared[("gq", i, b)]
            im[f"gk{i}"] = shared[("gk", i, b)]
            im[f"w0t{i}"] = shared[("w0t", i)]
            im[f"w1t{i}"] = shared[("w1t", i)]
            im[f"w2t{i}"] = shared[("w2t", i)]
            for j in range(3):
                im[f"b{j}_{i}"] = shared[(f"b{j}", i)]
        in_maps.append(im)
    return in_maps


def host_reduce(results):
    tot = np.float64(0.0)
    for r in results:
        for i, (C, HW) in enumerate(LAYERS):
            ssum = r[f"ssum{i}"].astype(np.float64)
            negm = r[f"negm{i}"].astype(np.float64)
            lp = r[f"lp{i}"].astype(np.float64)
            lse = np.log(ssum) - negm
            tot += lse.sum() - INVTAU * lp.sum()
    return np.float32(tot / (B * NUM_S))


_NC_CACHE = {}


def _get_nc():
    if "nc" not in _NC_CACHE:
        _NC_CACHE["nc"] = build_bass()
    return _NC_CACHE["nc"]


def kernel(**inputs):
    nc = _get_nc()
    in_maps = prep_in_maps(inputs)
    res = bass_utils.run_bass_kernel_spmd(nc, in_maps, core_ids=list(range(8)))
    return host_reduce(res.results)


if __name__ == "__main__":
    pass
